# revision 15
# baseline (speedup 1.0000x reference)
"""Trainium2 Bass kernel for the AdditiveModel reduction.

Computes out[y] = sum_{q,p} c[y,q] * a[y,q,p] * dot(lam[y,q,p,:], x[q,p,:])
with Y=16, Q=8, P=32, D=8192 (lam is 128 MiB -> memory-bound).

Sharding: one q per core (Q == 8 cores). Each core is fully independent and
produces a partial out[128, 16]; the host sums partials at gather time.

v2 (fp8): lam and x ship as FP8 E3M4 (4 mantissa bits; lam*4 and x*2 to sit
in e3m4's [0.25, 15.5] normal range) which halves HBM traffic vs fp16 and
keeps the measured end-to-end rel err ~0.9e-2 against the fp32 reference
(gate 2e-2). The 1/8 descale plus the c*a weights and the diagonal
projection mask are folded into one host-precomputed fp16 maskW.

PE: the D-axis dots are PSUM-accumulated matmuls lhsT=x[dchunk,p] (128x32),
rhs=lam[dchunk,(y,p)] (128x512). The 64 chunk-matmuls are spread over the
four 32-column groups of the PE array (tile_position=(0,32j)) so up to 4
matmuls stream concurrently -- M=32 only occupies a quarter of the array,
and column groups have independent XBUS streams. PSUM holds [128, 512];
group j's rows accumulate chunks i%4==j; the host collapses partitions.

DMA: lam streams over both HWDGE rings (SP, ACT) in 3 slabs per ring with
decreasing sizes (small tail to cut the last-slab completion latency).
10 total DMAs over 8 Tile completion lanes; the two reused lanes belong to
the early-finishing x transfers, so no issue stalls.
"""

from contextlib import ExitStack

import numpy as np

Y, Q, P, D = 16, 8, 32, 8192
NCORES = 8
KC = 128                 # contraction chunk (partition count)
DC = D // KC             # 64 d-chunks
YP = Y * P               # 512
S_LAM = 4.0              # lam scale into e3m4 range
S_X = 2.0                # x scale into e3m4 range
SLABS_A = [24, 20, 14, 6]   # sync-ring slab sizes (all 64 lam chunks)
SLABS_B = []                # act ring carries only x + mask + out
NA = sum(SLABS_A)
NB = sum(SLABS_B)

_CACHE = {}


def _build_nc():
    import concourse.bass as bass
    import concourse.mybir as mybir
    import concourse.tile as tile
    from concourse import bacc

    f32 = mybir.dt.float32
    f16 = mybir.dt.float16
    f8 = mybir.dt.float8e3
    nc = bacc.Bacc(None, target_bir_lowering=False)

    lamT = nc.declare_dram_parameter("lamT", [KC, DC * YP], f8, isOutput=False)
    xT = nc.declare_dram_parameter("xT", [KC, DC * P], f8, isOutput=False)
    maskW = nc.declare_dram_parameter("maskW", [KC, YP], f16, isOutput=False)
    out = nc.declare_dram_parameter("out", [KC, Y], f32, isOutput=True)

    with tile.TileContext(nc) as tc, ExitStack() as ctx:
        const = ctx.enter_context(tc.tile_pool(name="const", bufs=1))
        slab_pool = ctx.enter_context(
            tc.tile_pool(name="slab", bufs=len(SLABS_A) + len(SLABS_B))
        )
        psum_pool = ctx.enter_context(
            tc.tile_pool(name="psum", bufs=1, space=bass.MemorySpace.PSUM)
        )
        tailp = ctx.enter_context(tc.tile_pool(name="tail", bufs=1))

        # Single-queue lam stream: with two queues loaded, the SDMA engines
        # drain up to 64 large descriptors from one queue before switching
        # (megabyte-scale lurches, unpredictable finish order). One queue
        # alone sustains the same ~420 GB/s port ceiling deterministically.
        # The ACT ring carries only the small x/mask loads (done by ~9us)
        # and the final out store.
        x_sb = const.tile([KC, DC * P], f8)
        mask_sb = const.tile([KC, YP], f16)
        slab_ap = {}

        def emit_slab(lo, cnt):
            t = slab_pool.tile(
                [KC, cnt * YP], f8, tag="slab_a", name=f"slab_a{lo}"
            )
            nc.sync.dma_start(t[:], lamT[:, lo * YP:(lo + cnt) * YP])
            for k in range(cnt):
                slab_ap[lo + k] = t[:, k * YP:(k + 1) * YP]

        # x and mask lead the sync queue (small; their receipts clear long
        # before the first slab's), lam slabs stream behind them. The ACT
        # ring stays empty until the final out store, so the sync queue
        # owns all 16 SDMA ports for the whole stream.
        with tc.high_priority():
            nc.sync.dma_start(x_sb[:], xT[:])
            nc.sync.dma_start(mask_sb[:], maskW[:])
        lo = 0
        for cnt in SLABS_A:
            emit_slab(lo, cnt)
            lo += cnt

        seq = list(range(DC))

        # one PSUM bank per PE column group: start_tensor_calc zeroes at
        # bank granularity, so interleaved groups can't share a bank.
        projs = [
            psum_pool.tile([KC, YP], f32, tag=f"proj{j}", name=f"proj{j}")
            for j in range(4)
        ]
        counts = [0, 0, 0, 0]
        total = [sum(1 for i in range(DC) if i % 4 == j) for j in range(4)]
        for i, dc in enumerate(seq):
            j = i % 4
            counts[j] += 1
            nc.tensor.matmul(
                projs[j][32 * j:32 * (j + 1), :],
                x_sb[:, dc * P:(dc + 1) * P],
                slab_ap[dc],
                start=(counts[j] == 1),
                stop=(counts[j] == total[j]),
                tile_position=(0, 32 * j),
            )

        # tail: masked+weighted copy of PSUM, p-group reduce, DMA out.
        # t2 in bf16: halves the reduce's DVE read time (2x 16-bit mode);
        # the masked diag values only need ~8 mantissa bits here.
        bf16 = mybir.dt.bfloat16
        t2 = tailp.tile([KC, YP], bf16)
        for j in range(4):
            sl = slice(32 * j, 32 * (j + 1))
            nc.vector.tensor_mul(t2[sl, :], projs[j][sl, :], mask_sb[sl, :])
        s_t = tailp.tile([KC, Y], f32)
        nc.vector.reduce_sum(
            s_t[:],
            t2[:].rearrange("m (y p) -> m y p", p=P),
            axis=mybir.AxisListType.X,
        )
        nc.scalar.dma_start(out[:], s_t[:])

    nc.compile()
    return nc


def _shard_inputs(x, lam, a, c):
    """Per-core input maps. Slicing/layout/dtype transforms only."""
    import ml_dtypes

    f8 = ml_dtypes.float8_e3m4
    # maskW[r, y*32+p] = (r%32 == p) * c[y]a[y,p] / (S_LAM*S_X), tiled to 128
    eye = np.eye(P, dtype=np.float32)                      # [m, p]
    in_maps = []
    for q in range(NCORES):
        lam_q = lam[:, q]                                  # [Y, P, D]
        lamT = np.ascontiguousarray(
            lam_q.transpose(2, 0, 1).reshape(DC, KC, YP)
            .transpose(1, 0, 2).reshape(KC, DC * YP)
        )
        x_q = x[q]                                         # [P, D]
        xTn = np.ascontiguousarray(
            x_q.T.reshape(DC, KC, P).transpose(1, 0, 2).reshape(KC, DC * P)
        )
        w = (c[:, q][:, None] * a[:, q]) / (S_LAM * S_X)   # [Y, P]
        # blk[m, (y,p)] = (m==p) * w[y, p]
        blk = np.einsum('mp,yp->myp', eye, w).reshape(P, YP)
        mask_np = np.tile(blk, (KC // P, 1)).astype(np.float16)
        in_maps.append(
            {
                "lamT": (lamT * S_LAM).astype(f8),
                "xT": (xTn * S_X).astype(f8),
                "maskW": mask_np,
            }
        )
    return in_maps


def get_nc():
    key = (tuple(SLABS_A), tuple(SLABS_B))
    if key not in _CACHE:
        _CACHE[key] = _build_nc()
    return _CACHE[key]


def run(x, lam, a, c, trace=False, **spmd_kwargs):
    from concourse.bass_utils import run_bass_kernel_spmd

    nc = get_nc()
    in_maps = _shard_inputs(
        np.asarray(x, dtype=np.float32),
        np.asarray(lam, dtype=np.float32),
        np.asarray(a, dtype=np.float32),
        np.asarray(c, dtype=np.float32),
    )
    res = run_bass_kernel_spmd(
        nc, in_maps, core_ids=list(range(NCORES)), trace=trace, **spmd_kwargs
    )
    out = np.zeros((Y,), dtype=np.float32)
    for core_res in res.results:
        out += core_res["out"].reshape(KC, Y).sum(axis=0)
    return out, res


def kernel(x, lam, a, c):
    try:
        out, _ = run(x, lam, a, c, trace=False)
    except Exception:
        # one retry to ride out transient device errors
        out, _ = run(x, lam, a, c, trace=False)
    return out


# revision 17
# speedup vs baseline: 1.0568x; 1.0568x over previous
"""Trainium2 Bass kernel for the AdditiveModel reduction.

Computes out[y] = sum_{q,p} c[y,q] * a[y,q,p] * dot(lam[y,q,p,:], x[q,p,:])
with Y=16, Q=8, P=32, D=8192 (lam is 128 MiB -> memory-bound).

Sharding: one q per core (Q == 8 cores). Each core is fully independent and
produces a partial out[128, 16]; the host sums partials at gather time.

v2 (fp8): lam and x ship as FP8 E3M4 (4 mantissa bits; lam*4 and x*2 to sit
in e3m4's [0.25, 15.5] normal range) which halves HBM traffic vs fp16 and
keeps the measured end-to-end rel err ~0.9e-2 against the fp32 reference
(gate 2e-2). The 1/8 descale plus the c*a weights and the diagonal
projection mask are folded into one host-precomputed fp16 maskW.

PE: the D-axis dots are PSUM-accumulated matmuls lhsT=x[dchunk,p] (128x32),
rhs=lam[dchunk,(y,p)] (128x512). The 64 chunk-matmuls are spread over the
four 32-column groups of the PE array (tile_position=(0,32j)) so up to 4
matmuls stream concurrently -- M=32 only occupies a quarter of the array,
and column groups have independent XBUS streams. PSUM holds [128, 512];
group j's rows accumulate chunks i%4==j; the host collapses partitions.

DMA: lam streams over both HWDGE rings (SP, ACT) in 3 slabs per ring with
decreasing sizes (small tail to cut the last-slab completion latency).
10 total DMAs over 8 Tile completion lanes; the two reused lanes belong to
the early-finishing x transfers, so no issue stalls.
"""

from contextlib import ExitStack

import numpy as np

Y, Q, P, D = 16, 8, 32, 8192
NCORES = 8
KC = 128                 # contraction chunk (partition count)
DC = D // KC             # 64 d-chunks
YP = Y * P               # 512
S_LAM = 4.0              # lam scale into e3m4 range
S_X = 2.0                # x scale into e3m4 range
SLABS_A = [22, 20, 14, 8]   # sync-ring slab sizes (all 64 lam chunks)
SLABS_B = []                # act ring carries only x + mask + out
NA = sum(SLABS_A)
NB = sum(SLABS_B)

_CACHE = {}


def _build_nc():
    import concourse.bass as bass
    import concourse.mybir as mybir
    import concourse.tile as tile
    from concourse import bacc

    f32 = mybir.dt.float32
    f16 = mybir.dt.float16
    f8 = mybir.dt.float8e3
    nc = bacc.Bacc(None, target_bir_lowering=False)

    lamT = nc.declare_dram_parameter("lamT", [KC, DC * YP], f8, isOutput=False)
    xT = nc.declare_dram_parameter("xT", [KC, DC * P], f8, isOutput=False)
    maskW = nc.declare_dram_parameter("maskW", [KC, YP], f16, isOutput=False)
    out = nc.declare_dram_parameter("out", [KC, Y], f32, isOutput=True)

    with tile.TileContext(nc) as tc, ExitStack() as ctx:
        const = ctx.enter_context(tc.tile_pool(name="const", bufs=1))
        slab_pool = ctx.enter_context(
            tc.tile_pool(name="slab", bufs=len(SLABS_A) + len(SLABS_B))
        )
        psum_pool = ctx.enter_context(
            tc.tile_pool(name="psum", bufs=1, space=bass.MemorySpace.PSUM)
        )
        tailp = ctx.enter_context(tc.tile_pool(name="tail", bufs=1))

        # Single-queue lam stream: with two queues loaded, the SDMA engines
        # drain up to 64 large descriptors from one queue before switching
        # (megabyte-scale lurches, unpredictable finish order). One queue
        # alone sustains the same ~420 GB/s port ceiling deterministically.
        # The ACT ring carries only the small x/mask loads (done by ~9us)
        # and the final out store.
        x_sb = const.tile([KC, DC * P], f8)
        mask_sb = const.tile([KC, YP], f16)
        slab_ap = {}

        def emit_slab(lo, cnt):
            t = slab_pool.tile(
                [KC, cnt * YP], f8, tag="slab_a", name=f"slab_a{lo}"
            )
            nc.sync.dma_start(t[:], lamT[:, lo * YP:(lo + cnt) * YP])
            for k in range(cnt):
                slab_ap[lo + k] = t[:, k * YP:(k + 1) * YP]

        # x and mask ride the ACT ring (they dribble behind the sync ring's
        # port share but land well before their consumers need them); lam
        # owns the sync queue end to end.
        xh = DC // 2 * P
        with tc.high_priority():
            nc.scalar.dma_start(x_sb[:, 0:xh], xT[:, 0:xh])
            nc.scalar.dma_start(x_sb[:, xh:2 * xh], xT[:, xh:2 * xh])
            nc.scalar.dma_start(mask_sb[:], maskW[:])
        lo = 0
        for cnt in SLABS_A:
            emit_slab(lo, cnt)
            lo += cnt

        seq = list(range(DC))

        # one PSUM bank per PE column group: start_tensor_calc zeroes at
        # bank granularity, so interleaved groups can't share a bank.
        projs = [
            psum_pool.tile([KC, YP], f32, tag=f"proj{j}", name=f"proj{j}")
            for j in range(4)
        ]
        counts = [0, 0, 0, 0]
        total = [sum(1 for i in range(DC) if i % 4 == j) for j in range(4)]
        for i, dc in enumerate(seq):
            j = i % 4
            counts[j] += 1
            nc.tensor.matmul(
                projs[j][32 * j:32 * (j + 1), :],
                x_sb[:, dc * P:(dc + 1) * P],
                slab_ap[dc],
                start=(counts[j] == 1),
                stop=(counts[j] == total[j]),
                tile_position=(0, 32 * j),
            )

        # tail: masked+weighted copy of PSUM, p-group reduce, DMA out.
        # t2 in bf16: halves the reduce's DVE read time (2x 16-bit mode);
        # the masked diag values only need ~8 mantissa bits here.
        bf16 = mybir.dt.bfloat16
        t2 = tailp.tile([KC, YP], bf16)
        for j in range(4):
            sl = slice(32 * j, 32 * (j + 1))
            nc.vector.tensor_mul(t2[sl, :], projs[j][sl, :], mask_sb[sl, :])
        s_t = tailp.tile([KC, Y], f32)
        nc.vector.reduce_sum(
            s_t[:],
            t2[:].rearrange("m (y p) -> m y p", p=P),
            axis=mybir.AxisListType.X,
        )
        nc.scalar.dma_start(out[:], s_t[:])

    nc.compile()
    return nc


def _shard_inputs(x, lam, a, c):
    """Per-core input maps. Slicing/layout/dtype transforms only."""
    import ml_dtypes

    f8 = ml_dtypes.float8_e3m4
    # maskW[r, y*32+p] = (r%32 == p) * c[y]a[y,p] / (S_LAM*S_X), tiled to 128
    eye = np.eye(P, dtype=np.float32)                      # [m, p]
    in_maps = []
    for q in range(NCORES):
        lam_q = lam[:, q]                                  # [Y, P, D]
        lamT = np.ascontiguousarray(
            lam_q.transpose(2, 0, 1).reshape(DC, KC, YP)
            .transpose(1, 0, 2).reshape(KC, DC * YP)
        )
        x_q = x[q]                                         # [P, D]
        xTn = np.ascontiguousarray(
            x_q.T.reshape(DC, KC, P).transpose(1, 0, 2).reshape(KC, DC * P)
        )
        w = (c[:, q][:, None] * a[:, q]) / (S_LAM * S_X)   # [Y, P]
        # blk[m, (y,p)] = (m==p) * w[y, p]
        blk = np.einsum('mp,yp->myp', eye, w).reshape(P, YP)
        mask_np = np.tile(blk, (KC // P, 1)).astype(np.float16)
        in_maps.append(
            {
                "lamT": (lamT * S_LAM).astype(f8),
                "xT": (xTn * S_X).astype(f8),
                "maskW": mask_np,
            }
        )
    return in_maps


def get_nc():
    key = (tuple(SLABS_A), tuple(SLABS_B))
    if key not in _CACHE:
        _CACHE[key] = _build_nc()
    return _CACHE[key]


def run(x, lam, a, c, trace=False, **spmd_kwargs):
    from concourse.bass_utils import run_bass_kernel_spmd

    nc = get_nc()
    in_maps = _shard_inputs(
        np.asarray(x, dtype=np.float32),
        np.asarray(lam, dtype=np.float32),
        np.asarray(a, dtype=np.float32),
        np.asarray(c, dtype=np.float32),
    )
    res = run_bass_kernel_spmd(
        nc, in_maps, core_ids=list(range(NCORES)), trace=trace, **spmd_kwargs
    )
    out = np.zeros((Y,), dtype=np.float32)
    for core_res in res.results:
        out += core_res["out"].reshape(KC, Y).sum(axis=0)
    return out, res


def kernel(x, lam, a, c):
    try:
        out, _ = run(x, lam, a, c, trace=False)
    except Exception:
        # one retry to ride out transient device errors
        out, _ = run(x, lam, a, c, trace=False)
    return out


# revision 19
# speedup vs baseline: 1.1578x; 1.0955x over previous
"""Trainium2 Bass kernel for the AdditiveModel reduction.

Computes out[y] = sum_{q,p} c[y,q] * a[y,q,p] * dot(lam[y,q,p,:], x[q,p,:])
with Y=16, Q=8, P=32, D=8192 (lam is 128 MiB -> memory-bound).

Sharding: one q per core (Q == 8 cores). Each core is fully independent and
produces a partial out[128, 16]; the host sums partials at gather time.

v2 (fp8): lam and x ship as FP8 E3M4 (4 mantissa bits; lam*4 and x*2 to sit
in e3m4's [0.25, 15.5] normal range) which halves HBM traffic vs fp16 and
keeps the measured end-to-end rel err ~0.9e-2 against the fp32 reference
(gate 2e-2). The 1/8 descale plus the c*a weights and the diagonal
projection mask are folded into one host-precomputed fp16 maskW.

PE: the D-axis dots are PSUM-accumulated matmuls lhsT=x[dchunk,p] (128x32),
rhs=lam[dchunk,(y,p)] (128x512). The 64 chunk-matmuls are spread over the
four 32-column groups of the PE array (tile_position=(0,32j)) so up to 4
matmuls stream concurrently -- M=32 only occupies a quarter of the array,
and column groups have independent XBUS streams. PSUM holds [128, 512];
group j's rows accumulate chunks i%4==j; the host collapses partitions.

DMA: all of lam streams on the single SP HWDGE queue in 4 decreasing slabs
(a lone queue sustains the same ~420 GB/s port ceiling as two, without the
megabyte-scale lurching the SDMA packet-batch arbiter introduces when two
queues are loaded). x and the mask ride the ACT queue ahead of their
consumers; the final [128,16] partial goes out on ACT. 8 total DMAs over
8 Tile completion lanes -- no lane reuse, no issue stalls.
"""

from contextlib import ExitStack

import numpy as np

Y, Q, P, D = 16, 8, 32, 8192
NCORES = 8
KC = 128                 # contraction chunk (partition count)
DC = D // KC             # 64 d-chunks
YP = Y * P               # 512
S_LAM = 4.0              # lam scale into e3m4 range
S_X = 2.0                # x scale into e3m4 range
SLABS_A = [22, 20, 14, 8]   # sync-ring slab sizes (all 64 lam chunks)
SLABS_B = []                # act ring carries only x + mask + out
NA = sum(SLABS_A)
NB = sum(SLABS_B)

_CACHE = {}


def _build_nc():
    import concourse.bass as bass
    import concourse.mybir as mybir
    import concourse.tile as tile
    from concourse import bacc

    f32 = mybir.dt.float32
    f16 = mybir.dt.float16
    f8 = mybir.dt.float8e3
    nc = bacc.Bacc(None, target_bir_lowering=False)

    lamT = nc.declare_dram_parameter("lamT", [KC, DC * YP], f8, isOutput=False)
    xT = nc.declare_dram_parameter("xT", [KC, DC * P], f8, isOutput=False)
    maskW = nc.declare_dram_parameter("maskW", [KC, YP], f16, isOutput=False)
    out = nc.declare_dram_parameter("out", [KC, Y], f32, isOutput=True)

    with tile.TileContext(nc) as tc, ExitStack() as ctx:
        const = ctx.enter_context(tc.tile_pool(name="const", bufs=1))
        slab_pool = ctx.enter_context(
            tc.tile_pool(name="slab", bufs=len(SLABS_A) + len(SLABS_B))
        )
        psum_pool = ctx.enter_context(
            tc.tile_pool(name="psum", bufs=1, space=bass.MemorySpace.PSUM)
        )
        tailp = ctx.enter_context(tc.tile_pool(name="tail", bufs=1))

        # Single-queue lam stream: with two queues loaded, the SDMA engines
        # drain up to 64 large descriptors from one queue before switching
        # (megabyte-scale lurches, unpredictable finish order). One queue
        # alone sustains the same ~420 GB/s port ceiling deterministically.
        # The ACT ring carries only the small x/mask loads (done by ~9us)
        # and the final out store.
        x_sb = const.tile([KC, DC * P], f8)
        mask_sb = const.tile([KC, YP], f16)
        slab_ap = {}

        def emit_slab(lo, cnt):
            t = slab_pool.tile(
                [KC, cnt * YP], f8, tag="slab_a", name=f"slab_a{lo}"
            )
            nc.sync.dma_start(t[:], lamT[:, lo * YP:(lo + cnt) * YP])
            for k in range(cnt):
                slab_ap[lo + k] = t[:, k * YP:(k + 1) * YP]

        # x and mask ride the ACT ring (they dribble behind the sync ring's
        # port share but land well before their consumers need them); lam
        # owns the sync queue end to end.
        xh = DC // 2 * P
        with tc.high_priority():
            nc.scalar.dma_start(x_sb[:, 0:xh], xT[:, 0:xh])
            nc.scalar.dma_start(x_sb[:, xh:2 * xh], xT[:, xh:2 * xh])
            nc.scalar.dma_start(mask_sb[:], maskW[:])
        lo = 0
        for cnt in SLABS_A:
            emit_slab(lo, cnt)
            lo += cnt

        seq = list(range(DC))

        # one PSUM bank per PE column group: start_tensor_calc zeroes at
        # bank granularity, so interleaved groups can't share a bank.
        projs = [
            psum_pool.tile([KC, YP], f32, tag=f"proj{j}", name=f"proj{j}")
            for j in range(4)
        ]
        counts = [0, 0, 0, 0]
        total = [sum(1 for i in range(DC) if i % 4 == j) for j in range(4)]
        for i, dc in enumerate(seq):
            j = i % 4
            counts[j] += 1
            nc.tensor.matmul(
                projs[j][32 * j:32 * (j + 1), :],
                x_sb[:, dc * P:(dc + 1) * P],
                slab_ap[dc],
                start=(counts[j] == 1),
                stop=(counts[j] == total[j]),
                tile_position=(0, 32 * j),
            )

        # tail: masked+weighted copy of PSUM, p-group reduce, DMA out.
        # (t2 must stay fp32: a bf16 t2 measured ~2.5us SLOWER -- the
        # fp32->bf16 conversion breaks the DVE fast path.)
        t2 = tailp.tile([KC, YP], f32)
        for j in range(4):
            sl = slice(32 * j, 32 * (j + 1))
            nc.vector.tensor_mul(t2[sl, :], projs[j][sl, :], mask_sb[sl, :])
        s_t = tailp.tile([KC, Y], f32)
        nc.vector.reduce_sum(
            s_t[:],
            t2[:].rearrange("m (y p) -> m y p", p=P),
            axis=mybir.AxisListType.X,
        )
        nc.scalar.dma_start(out[:], s_t[:])

    nc.compile()
    return nc


def _shard_inputs(x, lam, a, c):
    """Per-core input maps. Slicing/layout/dtype transforms only."""
    import ml_dtypes

    f8 = ml_dtypes.float8_e3m4
    # maskW[r, y*32+p] = (r%32 == p) * c[y]a[y,p] / (S_LAM*S_X), tiled to 128
    eye = np.eye(P, dtype=np.float32)                      # [m, p]
    in_maps = []
    for q in range(NCORES):
        lam_q = lam[:, q]                                  # [Y, P, D]
        lamT = np.ascontiguousarray(
            lam_q.transpose(2, 0, 1).reshape(DC, KC, YP)
            .transpose(1, 0, 2).reshape(KC, DC * YP)
        )
        x_q = x[q]                                         # [P, D]
        xTn = np.ascontiguousarray(
            x_q.T.reshape(DC, KC, P).transpose(1, 0, 2).reshape(KC, DC * P)
        )
        w = (c[:, q][:, None] * a[:, q]) / (S_LAM * S_X)   # [Y, P]
        # blk[m, (y,p)] = (m==p) * w[y, p]
        blk = np.einsum('mp,yp->myp', eye, w).reshape(P, YP)
        mask_np = np.tile(blk, (KC // P, 1)).astype(np.float16)
        in_maps.append(
            {
                "lamT": (lamT * S_LAM).astype(f8),
                "xT": (xTn * S_X).astype(f8),
                "maskW": mask_np,
            }
        )
    return in_maps


def get_nc():
    key = (tuple(SLABS_A), tuple(SLABS_B))
    if key not in _CACHE:
        _CACHE[key] = _build_nc()
    return _CACHE[key]


def run(x, lam, a, c, trace=False, **spmd_kwargs):
    from concourse.bass_utils import run_bass_kernel_spmd

    nc = get_nc()
    in_maps = _shard_inputs(
        np.asarray(x, dtype=np.float32),
        np.asarray(lam, dtype=np.float32),
        np.asarray(a, dtype=np.float32),
        np.asarray(c, dtype=np.float32),
    )
    res = run_bass_kernel_spmd(
        nc, in_maps, core_ids=list(range(NCORES)), trace=trace, **spmd_kwargs
    )
    out = np.zeros((Y,), dtype=np.float32)
    for core_res in res.results:
        out += core_res["out"].reshape(KC, Y).sum(axis=0)
    return out, res


def kernel(x, lam, a, c):
    try:
        out, _ = run(x, lam, a, c, trace=False)
    except Exception:
        # one retry to ride out transient device errors
        out, _ = run(x, lam, a, c, trace=False)
    return out
